# revision 1
# baseline (speedup 1.0000x reference)
"""Multi-head self-attention with RoPE on 8 Trainium2 NeuronCores.

Sharding: tensor-parallel over heads (16 heads / 8 cores = 2 heads per core).
Each core computes Q/K/V projections for its 2 heads over all 4 batches
(fp8 DoubleRow matmuls, weights pre-scaled x16 with the scale folded into the
exp scale and the 1/l normalization), causal flash-style attention (scores
computed transposed; no on-chip transposes), and a partial output projection
over its 128 rows of Wo. Host sums the 8 bf16 partial outputs in f32.

Self-contained: hardcodes all shapes from the problem spec.
"""

import numpy as np
import ml_dtypes

BF16 = ml_dtypes.bfloat16
FP8 = ml_dtypes.float8_e4m3  # unused; fp8 busts the 2e-2 gate

B, S, DM = 4, 2048, 1024
H, DH = 16, 64
NCORES = 8
HPC = H // NCORES  # 2 heads per core
DL = HPC * DH  # 128 local head dims per core
SB = 512  # q-block width
NSB = S // SB  # 4
NKT = S // 128  # 16 k-tiles per batch
LN_THETA = float(np.log(10000.0))
WSCALE = 1.0  # no pre-scale in the bf16 pipeline
EXP_SCALE = 0.125 / (WSCALE * WSCALE)

CFG = {
    "S_BUFS": 2,
    "Y_BUFS": 2,
    "PS_BUFS": 2,
    "DEFER": True,
    "QK_BUFS": 2,
    "XT_BUFS": 2,
    "V_BUFS": 2,
    "L_BUFS": 2,
    "P_BUFS": 6,
    "R_BUFS": 3,
    "O_BUFS": 2,
}


def _build_nc(reps=1):
    import concourse.bass as bass
    import concourse.tile as tile
    import concourse.mybir as mybir
    from concourse import bacc

    dt = mybir.dt
    F32 = dt.float32
    BF = dt.bfloat16
    F8 = dt.float8e4
    AF = mybir.ActivationFunctionType
    DR = mybir.MatmulPerfMode.DoubleRow

    nc = bacc.Bacc("TRN2", target_bir_lowering=False, debug=False)

    xt_d = nc.dram_tensor("xt", [B, DM, S], BF, kind="ExternalInput").ap()
    wq_d = nc.dram_tensor("wqt", [DM, DL], BF, kind="ExternalInput").ap()
    wk_d = nc.dram_tensor("wkt", [DM, DL], BF, kind="ExternalInput").ap()
    wv_d = nc.dram_tensor("wvt", [DM, DL], BF, kind="ExternalInput").ap()
    wo_d = nc.dram_tensor("wot", [DL, DM], BF, kind="ExternalInput").ap()
    pos_d = nc.dram_tensor("pos", [1, S], dt.int32, kind="ExternalInput").ap()
    out_d = nc.dram_tensor("outp", [B, S, DM], BF, kind="ExternalOutput").ap()

    with tile.TileContext(nc) as tc:
        import contextlib

        ctx = contextlib.ExitStack()
        with ctx:
            # ---------------- pools ----------------
            consts = ctx.enter_context(tc.tile_pool(name="consts", bufs=1))
            xt_p = ctx.enter_context(tc.tile_pool(name="xt", bufs=CFG["XT_BUFS"]))
            qk_p = ctx.enter_context(tc.tile_pool(name="qk", bufs=CFG["QK_BUFS"]))
            rope_p = ctx.enter_context(tc.tile_pool(name="rope", bufs=CFG["R_BUFS"]))
            setup_p = ctx.enter_context(tc.tile_pool(name="setup", bufs=1))
            v_p = ctx.enter_context(tc.tile_pool(name="v", bufs=CFG["V_BUFS"]))
            p_p = ctx.enter_context(tc.tile_pool(name="p", bufs=CFG["P_BUFS"]))
            lin_p = ctx.enter_context(tc.tile_pool(name="lin", bufs=CFG["L_BUFS"]))
            outs_p = ctx.enter_context(tc.tile_pool(name="outs", bufs=CFG["O_BUFS"]))
            ps2_p = ctx.enter_context(
                tc.tile_pool(name="ps2", bufs=1, space="PSUM")
            )
            outp_p = ctx.enter_context(
                tc.tile_pool(name="outp", bufs=1, space="PSUM")
            )
            s_p = ctx.enter_context(
                tc.tile_pool(name="s", bufs=CFG["S_BUFS"], space="PSUM")
            )
            y_p = ctx.enter_context(
                tc.tile_pool(name="y", bufs=CFG["Y_BUFS"], space="PSUM")
            )

            # ---------------- weights -> SBUF ----------------
            wq_sb = consts.tile([128, 8, DL], BF, tag="wq")
            wk_sb = consts.tile([128, 8, DL], BF, tag="wk")
            wv_sb = consts.tile([128, 8, DL], BF, tag="wv")
            wo_sb = consts.tile([128, DM], BF, tag="wo")
            nc.sync.dma_start(out=wq_sb, in_=wq_d.rearrange("(t p) d -> p t d", p=128))
            nc.sync.dma_start(out=wk_sb, in_=wk_d.rearrange("(t p) d -> p t d", p=128))
            nc.sync.dma_start(out=wv_sb, in_=wv_d.rearrange("(t p) d -> p t d", p=128))
            nc.sync.dma_start(out=wo_sb, in_=wo_d)

            # ---------------- cos/sin tables ----------------
            # invf row [1, 32]: exp(-j * 2*ln(theta)/64)
            invf_i = consts.tile([1, 32], dt.int32, tag="invf_i")
            nc.gpsimd.iota(invf_i, pattern=[[1, 32]], base=0, channel_multiplier=0)
            invf_f = consts.tile([1, 32], F32, tag="invf_f")
            nc.vector.tensor_copy(invf_f, invf_i)
            invf = consts.tile([1, 32], F32, tag="invf")
            nc.scalar.activation(invf, invf_f, AF.Exp, scale=-(2.0 * LN_THETA / 64.0))

            pos_i = consts.tile([1, S], dt.int32, tag="pos_i")
            nc.sync.dma_start(out=pos_i, in_=pos_d)
            pos_f = consts.tile([1, S], F32, tag="pos_f")
            nc.vector.tensor_copy(pos_f, pos_i)

            sin32 = consts.tile([32, S], BF, tag="sin32")
            nsin32 = consts.tile([32, S], BF, tag="nsin32")
            cos32 = consts.tile([32, S], BF, tag="cos32")
            # Sin LUT needs args in [-pi, pi]: Cody-Waite range reduction.
            # HW f32->i32 conversion rounds to nearest; CoreSim truncates.
            # The is_gt fix-up makes the result exact under both (args >= 0).
            INV2PI = float(1.0 / (2.0 * np.pi))
            C1 = 6.28125
            C2 = float(2.0 * np.pi - 6.28125)
            TWO_PI = float(2.0 * np.pi)

            def reduce_to_pi(x):
                # x >= 0 (SBUF or PSUM AP) -> SBUF f32 in [-pi, pi]
                t = setup_p.tile([32, SB], F32, tag="rr_t")
                nc.vector.tensor_scalar_mul(t, x, INV2PI)
                ri = setup_p.tile([32, SB], dt.int32, tag="rr_i")
                nc.vector.tensor_copy(ri, t)
                rf = setup_p.tile([32, SB], F32, tag="rr_f")
                nc.vector.tensor_copy(rf, ri)
                a1 = setup_p.tile([32, SB], F32, tag="rr_a1")
                nc.vector.scalar_tensor_tensor(
                    a1, rf, -C1, x,
                    op0=mybir.AluOpType.mult, op1=mybir.AluOpType.add,
                )
                a2 = setup_p.tile([32, SB], F32, tag="rr_a2")
                nc.vector.scalar_tensor_tensor(
                    a2, rf, -C2, a1,
                    op0=mybir.AluOpType.mult, op1=mybir.AluOpType.add,
                )
                over = setup_p.tile([32, SB], F32, tag="rr_ov")
                nc.vector.tensor_scalar(
                    over, a2, float(np.pi), None, op0=mybir.AluOpType.is_gt
                )
                a3 = setup_p.tile([32, SB], F32, tag="rr_a3")
                nc.vector.scalar_tensor_tensor(
                    a3, over, -TWO_PI, a2,
                    op0=mybir.AluOpType.mult, op1=mybir.AluOpType.add,
                )
                return a3

            for cchunk in range(NSB):
                csl = slice(cchunk * SB, (cchunk + 1) * SB)
                ang = outp_p.tile([128, SB], F32, tag="oproj")
                # angles = outer(invf, pos) via K=1 fp32 matmul
                nc.tensor.matmul(
                    ang[0:32, :], lhsT=invf, rhs=pos_f[:, csl], start=True, stop=True
                )
                angv = ang[0:32, :]
                a_s = reduce_to_pi(angv)
                nc.scalar.activation(sin32[:, csl], a_s, AF.Sin)
                nc.scalar.activation(nsin32[:, csl], a_s, AF.Sin, scale=-1.0)
                shifted = setup_p.tile([32, SB], F32, tag="rr_sh")
                nc.vector.tensor_scalar_add(shifted, angv, float(np.pi / 2))
                a_c = reduce_to_pi(shifted)
                nc.scalar.activation(cos32[:, csl], a_c, AF.Sin)
            # head-major pair layout: rows [a_h0, b_h0, a_h1, b_h1]
            # cosD [128, S] = cos x4 ; sinPM [128, S] = [-sin; +sin; -sin; +sin]
            cosD = consts.tile([128, S], BF, tag="cosD")
            sinPM = consts.tile([128, S], BF, tag="sinPM")
            for r in range(4):
                nc.sync.dma_start(out=cosD[32 * r : 32 * (r + 1), :], in_=cos32)
            nc.sync.dma_start(out=sinPM[0:32, :], in_=nsin32)
            nc.sync.dma_start(out=sinPM[32:64, :], in_=sin32)
            nc.sync.dma_start(out=sinPM[64:96, :], in_=nsin32)
            nc.sync.dma_start(out=sinPM[96:128, :], in_=sin32)

            # ---------------- causal triangle mask [128, 2, 128] ----------------
            # applied to the first 128-col strip of the exp'd region of
            # diagonal tiles: keep iff f' - p >= 0
            tri = consts.tile([128, 2, 128], BF, tag="tri")
            nc.gpsimd.memset(tri, 1.0)
            nc.gpsimd.affine_select(
                out=tri,
                in_=tri,
                compare_op=mybir.AluOpType.is_ge,
                fill=0.0,
                base=0,
                pattern=[[0, 2], [1, 128]],
                channel_multiplier=-1,
            )

            # ---------------- main loop over batches ----------------
            pending = None
            for b in [bb for _ in range(reps) for bb in range(B)]:
                if pending is not None:
                    norm_and_outproj(*pending)
                    pending = None
                # x^T for this batch: one DMA, [128, 8, S] fp8
                xt_t = xt_p.tile([128, 8, S], BF, tag="xt")
                for xc in range(4):
                    xsl = slice(xc * SB, (xc + 1) * SB)
                    nc.sync.dma_start(
                        out=xt_t[:, :, xsl],
                        in_=xt_d[b].rearrange("(t p) s -> p t s", p=128)[:, :, xsl],
                    )

                # ---- V projection (natural [s, d] layout + ones cols) ----
                v_sb = v_p.tile([128, NKT, 130], BF, tag="v")
                nc.vector.memset(v_sb[:, :, 64:65], 1.0)
                nc.vector.memset(v_sb[:, :, 129:130], 1.0)
                for kt in range(NKT):
                    vps = ps2_p.tile([128, SB], F32, tag="proj")
                    for mt in range(8):
                        nc.tensor.matmul(
                            vps[:, 0:128],
                            lhsT=xt_t[:, mt, 128 * kt : 128 * (kt + 1)],
                            rhs=wv_sb[:, mt, :],
                            start=(mt == 0),
                            stop=(mt == 7),
                        )
                    nc.vector.tensor_copy(
                        v_sb[:, kt, 0:130].rearrange("p (a c) -> p a c", a=2)[
                            :, :, 0:64
                        ],
                        vps[:, 0:128].rearrange("p (a c) -> p a c", a=2),
                    )

                # ---- Q^T / K^T projections + RoPE ----
                qr = qk_p.tile([128, S], BF, tag="qr")
                kr = qk_p.tile([128, S], BF, tag="kr")
                for (w_sb, dst) in ((wq_sb, qr), (wk_sb, kr)):
                    for sb_i in range(NSB):
                        ssl = slice(sb_i * SB, (sb_i + 1) * SB)
                        tps = ps2_p.tile([128, SB], F32, tag="proj")
                        for mt in range(8):
                            nc.tensor.matmul(
                                tps,
                                lhsT=w_sb[:, mt, :],
                                rhs=xt_t[:, mt, ssl],
                                start=(mt == 0),
                                stop=(mt == 7),
                            )
                        tsb = rope_p.tile([128, SB], BF, tag="tsb")
                        nc.scalar.activation(tsb, tps, AF.Copy)
                        # partner swap within each head: a<->b 32-blocks
                        tswap = rope_p.tile([128, SB], BF, tag="tswap")
                        for h0 in (0, 64):
                            nc.sync.dma_start(
                                out=tswap[h0 : h0 + 32, :],
                                in_=tsb[h0 + 32 : h0 + 64, :],
                            )
                            nc.sync.dma_start(
                                out=tswap[h0 + 32 : h0 + 64, :],
                                in_=tsb[h0 : h0 + 32, :],
                            )
                        tcos = rope_p.tile([128, SB], BF, tag="tcos")
                        nc.vector.tensor_mul(tcos, tsb, cosD[:, ssl])
                        tsin = rope_p.tile([128, SB], BF, tag="tsin")
                        nc.gpsimd.tensor_mul(tsin, tswap, sinPM[:, ssl])
                        nc.vector.tensor_add(dst[:, ssl], tcos, tsin)

                # ---- attention per q-block ----
                def norm_and_outproj(b, qb, y0, y1):
                    # l rows to partition-0 tiles (gpsimd/custom-DVE ucode
                    # ignores AP partition bases), broadcast raw l on gpsimd,
                    # then approx-reciprocal across 64 lanes
                    l0t = lin_p.tile([1, SB], F32, tag="l0t")
                    l1t = lin_p.tile([1, SB], F32, tag="l1t")
                    nc.vector.tensor_copy(l0t, y0[64:65, :])
                    nc.vector.tensor_copy(l1t, y1[64:65, :])
                    lb0r = lin_p.tile([64, SB], F32, tag="lb0r")
                    lb1r = lin_p.tile([64, SB], F32, tag="lb1r")
                    nc.gpsimd.partition_broadcast(lb0r, l0t)
                    nc.gpsimd.partition_broadcast(lb1r, l1t)
                    with nc.allow_low_precision("softmax 1/l"):
                        nc.vector.reciprocal_approx_fast(lb0r, lb0r)
                        nc.vector.reciprocal_approx_fast(lb1r, lb1r)
                    lb0, lb1 = lb0r, lb1r
                    ysb = lin_p.tile([128, SB], BF, tag="ysb")
                    # ysb = (y/16) * (1/l)  (the /16 cancels the V weight scale)
                    nc.vector.scalar_tensor_tensor(
                        ysb[0:64, :], y0[0:64, :], 1.0 / WSCALE, lb0,
                        op0=mybir.AluOpType.mult, op1=mybir.AluOpType.mult,
                    )
                    nc.vector.scalar_tensor_tensor(
                        ysb[64:128, :], y1[0:64, :], 1.0 / WSCALE, lb1,
                        op0=mybir.AluOpType.mult, op1=mybir.AluOpType.mult,
                    )

                    # ---- output projection for this q-block ----
                    osb = outs_p.tile([128, 4, DM], BF, tag="osb")
                    for jj in range(4):
                        for mc in range(2):
                            msl = slice(512 * mc, 512 * (mc + 1))
                            ops = outp_p.tile([128, SB], F32, tag="oproj")
                            nc.tensor.matmul(
                                ops,
                                lhsT=ysb[:, 128 * jj : 128 * (jj + 1)],
                                rhs=wo_sb[:, msl],
                                start=True,
                                stop=True,
                            )
                            nc.vector.tensor_copy(osb[:, jj, msl], ops)
                    nc.sync.dma_start(
                        out=out_d[b, qb * SB : (qb + 1) * SB, :].rearrange(
                            "(j p) m -> p j m", p=128
                        ),
                        in_=osb,
                    )

                for qb in range(NSB):
                    qsl = slice(qb * SB, (qb + 1) * SB)
                    nkb = 4 * (qb + 1)
                    y0 = y_p.tile([128, SB], F32, tag="y")
                    y1 = y_p.tile([128, SB], F32, tag="y")
                    for kb in range(nkb):
                        ksl = slice(128 * kb, 128 * (kb + 1))
                        s_t = s_p.tile([128, 2, SB], F32, tag="s")
                        nc.tensor.matmul(
                            s_t[:, 0, :], lhsT=kr[0:64, ksl], rhs=qr[0:64, qsl],
                            start=True, stop=True,
                        )
                        nc.tensor.matmul(
                            s_t[:, 1, :], lhsT=kr[64:128, ksl], rhs=qr[64:128, qsl],
                            start=True, stop=True,
                        )
                        p_t = p_p.tile([128, 2, SB], BF, tag="p")
                        j = kb - 4 * qb
                        if j < 0:
                            # dense tile: exp everything
                            nc.scalar.activation(p_t, s_t, AF.Exp, scale=EXP_SCALE)
                        else:
                            # diagonal tile: cols < 128j are fully masked
                            if j > 0:
                                nc.gpsimd.memset(p_t[:, :, 0 : 128 * j], 0.0)
                            nc.scalar.activation(
                                p_t[:, :, 128 * j :],
                                s_t[:, :, 128 * j :],
                                AF.Exp,
                                scale=EXP_SCALE,
                            )
                            # triangular boundary strip
                            nc.vector.tensor_mul(
                                p_t[:, :, 128 * j : 128 * (j + 1)],
                                p_t[:, :, 128 * j : 128 * (j + 1)],
                                tri,
                            )
                        nc.tensor.matmul(
                            y0[0:65, :],
                            lhsT=v_sb[:, kb, 0:65],
                            rhs=p_t[:, 0, :],
                            start=(kb == 0),
                            stop=(kb == nkb - 1),
                        )
                        nc.tensor.matmul(
                            y1[0:65, :],
                            lhsT=v_sb[:, kb, 65:130],
                            rhs=p_t[:, 1, :],
                            start=(kb == 0),
                            stop=(kb == nkb - 1),
                        )
                    if CFG["DEFER"]:
                        if pending is not None:
                            norm_and_outproj(*pending)
                        pending = (b, qb, y0, y1)
                    else:
                        norm_and_outproj(b, qb, y0, y1)

            if pending is not None:
                norm_and_outproj(*pending)

    nc.compile()
    return nc


_NC_CACHE = {}


def get_nc(reps=1):
    if reps not in _NC_CACHE:
        _NC_CACHE[reps] = _build_nc(reps)
    return _NC_CACHE[reps]


def make_in_maps(x, token_positions, Wq, Wk, Wv, Wo):
    x = np.asarray(x, dtype=np.float32)
    Wq, Wk, Wv, Wo = (np.asarray(w, dtype=np.float32) for w in (Wq, Wk, Wv, Wo))
    pos = np.ascontiguousarray(
        np.asarray(token_positions, dtype=np.int32).reshape(1, S)
    )
    xt = np.ascontiguousarray(x.transpose(0, 2, 1)).astype(BF16)
    in_maps = []
    for c in range(NCORES):
        h0, h1 = 2 * c, 2 * c + 1
        # head-major pair layout: [a_h0(32), b_h0(32), a_h1(32), b_h1(32)]
        rows = np.concatenate(
            [
                64 * h0 + np.arange(0, 64, 2),
                64 * h0 + np.arange(1, 64, 2),
                64 * h1 + np.arange(0, 64, 2),
                64 * h1 + np.arange(1, 64, 2),
            ]
        )
        in_maps.append(
            {
                "xt": xt,
                "wqt": np.ascontiguousarray(Wq[rows, :].T).astype(BF16),
                "wkt": np.ascontiguousarray(Wk[rows, :].T).astype(BF16),
                "wvt": np.ascontiguousarray(
                    Wv[128 * c : 128 * (c + 1), :].T
                ).astype(BF16),
                "wot": np.ascontiguousarray(
                    Wo[:, 128 * c : 128 * (c + 1)].T
                ).astype(BF16),
                "pos": pos,
            }
        )
    return in_maps


def kernel(x, token_positions, Wq, Wk, Wv, Wo):
    from concourse.bass_utils import run_bass_kernel_spmd

    nc = get_nc()
    in_maps = make_in_maps(x, token_positions, Wq, Wk, Wv, Wo)
    res = run_bass_kernel_spmd(nc, in_maps, core_ids=list(range(NCORES)))
    out = np.zeros((B, S, DM), np.float32)
    for r in res.results:
        out += r["outp"].astype(np.float32)
    return out



# revision 7
# speedup vs baseline: 1.1119x; 1.1119x over previous
"""Multi-head self-attention with RoPE on 8 Trainium2 NeuronCores.

Sharding: 2-way tensor parallel over heads x 4-way data parallel over batch.
Core c handles batch (c % 4) and head group (c // 4) = 8 heads = 4 head-pairs.
Each core computes Q/K/V projections for its 8 heads on its batch, causal
flash-style attention per head-pair (scores computed transposed, row-tiled
across PE quadrants; softmax denominator via a ones-column in V), and a
partial output projection over its 512 rows of Wo. Host sums 2 partials per
batch in f32.

vs the 2-heads x 4-batches sharding this cuts the per-core partial-output
volume (PSUM drain + DMA) by 4x for the same PE work.

Self-contained: hardcodes all shapes from the problem spec.
"""

import numpy as np
import ml_dtypes

BF16 = ml_dtypes.bfloat16

B, S, DM = 4, 2048, 1024
H, DH = 16, 64
NCORES = 8
NPAIR = 4  # head-pairs per core
DL = NPAIR * 2 * DH  # 512 local head dims per core
SB = 512  # q-block width
NSB = S // SB  # 4
NKT = S // 128  # 16 k-tiles
LN_THETA = float(np.log(10000.0))
EXP_SCALE = 0.125

CFG = {
    "S_BUFS": 2,
    "Y_BUFS": 2,
    "PS_BUFS": 2,
    "QK_BUFS": 1,
    "V_BUFS": 1,
    "L_BUFS": 1,
    "P_BUFS": 5,
    "R_BUFS": 3,
    "O_BUFS": 2,
}


def _build_nc(reps=1):
    import concourse.bass as bass
    import concourse.tile as tile
    import concourse.mybir as mybir
    from concourse import bacc

    dt = mybir.dt
    F32 = dt.float32
    BF = dt.bfloat16
    AF = mybir.ActivationFunctionType

    nc = bacc.Bacc("TRN2", target_bir_lowering=False, debug=False)

    xt_d = nc.dram_tensor("xt", [DM // 128, 128, S], BF, kind="ExternalInput").ap()
    wq_d = nc.dram_tensor("wqt", [DM, DL], BF, kind="ExternalInput").ap()
    wk_d = nc.dram_tensor("wkt", [DM, DL], BF, kind="ExternalInput").ap()
    wv_d = nc.dram_tensor("wvt", [DM, DL], BF, kind="ExternalInput").ap()
    wo_d = nc.dram_tensor("wot", [DL, DM], BF, kind="ExternalInput").ap()
    pos_d = nc.dram_tensor("pos", [1, S], dt.int32, kind="ExternalInput").ap()
    out_d = nc.dram_tensor("outp", [S, DM], BF, kind="ExternalOutput").ap()

    with tile.TileContext(nc) as tc:
        import contextlib

        ctx = contextlib.ExitStack()
        with ctx:
            # ---------------- pools ----------------
            consts = ctx.enter_context(tc.tile_pool(name="consts", bufs=1))
            xt_p = ctx.enter_context(tc.tile_pool(name="xt", bufs=1))
            qk_p = ctx.enter_context(tc.tile_pool(name="qk", bufs=CFG["QK_BUFS"]))
            rope_p = ctx.enter_context(tc.tile_pool(name="rope", bufs=CFG["R_BUFS"]))
            setup_p = ctx.enter_context(tc.tile_pool(name="setup", bufs=1))
            v_p = ctx.enter_context(tc.tile_pool(name="v", bufs=CFG["V_BUFS"]))
            p_p = ctx.enter_context(tc.tile_pool(name="p", bufs=CFG["P_BUFS"]))
            lin_p = ctx.enter_context(tc.tile_pool(name="lin", bufs=CFG["L_BUFS"]))
            outs_p = ctx.enter_context(tc.tile_pool(name="outs", bufs=CFG["O_BUFS"]))
            # shared [128, 512] f32 PSUM pool: projections, angle outer-
            # products, and output projection (phases don't overlap much)
            ps_p = ctx.enter_context(
                tc.tile_pool(name="ps", bufs=CFG["PS_BUFS"], space="PSUM")
            )
            s_p = ctx.enter_context(
                tc.tile_pool(name="s", bufs=CFG["S_BUFS"], space="PSUM")
            )
            y_p = ctx.enter_context(
                tc.tile_pool(name="y", bufs=CFG["Y_BUFS"], space="PSUM")
            )

            # ---------------- weights -> SBUF ----------------
            wq_sb = consts.tile([128, 8, DL], BF, tag="wq")
            wk_sb = consts.tile([128, 8, DL], BF, tag="wk")
            wv_sb = consts.tile([128, 8, DL], BF, tag="wv")
            wo_sb = consts.tile([128, NPAIR, DM], BF, tag="wo")
            nc.sync.dma_start(out=wq_sb, in_=wq_d.rearrange("(t p) d -> p t d", p=128))
            nc.sync.dma_start(out=wk_sb, in_=wk_d.rearrange("(t p) d -> p t d", p=128))
            nc.sync.dma_start(out=wv_sb, in_=wv_d.rearrange("(t p) d -> p t d", p=128))
            nc.sync.dma_start(out=wo_sb, in_=wo_d.rearrange("(t p) m -> p t m", p=128))

            # ---------------- cos/sin tables ----------------
            # invf row [1, 32]: exp(-j * 2*ln(theta)/64)
            invf_i = consts.tile([1, 32], dt.int32, tag="invf_i")
            nc.gpsimd.iota(invf_i, pattern=[[1, 32]], base=0, channel_multiplier=0)
            invf_f = consts.tile([1, 32], F32, tag="invf_f")
            nc.vector.tensor_copy(invf_f, invf_i)
            invf = consts.tile([1, 32], F32, tag="invf")
            nc.scalar.activation(invf, invf_f, AF.Exp, scale=-(2.0 * LN_THETA / 64.0))

            pos_i = consts.tile([1, S], dt.int32, tag="pos_i")
            nc.sync.dma_start(out=pos_i, in_=pos_d)
            pos_f = consts.tile([1, S], F32, tag="pos_f")
            nc.vector.tensor_copy(pos_f, pos_i)

            sin32 = consts.tile([32, S], BF, tag="sin32")
            nsin32 = consts.tile([32, S], BF, tag="nsin32")
            cos32 = consts.tile([32, S], BF, tag="cos32")
            # Sin LUT needs args in [-pi, pi]: Cody-Waite range reduction.
            # HW f32->i32 conversion rounds to nearest; CoreSim truncates.
            # The is_gt fix-up makes the result exact under both (args >= 0).
            INV2PI = float(1.0 / (2.0 * np.pi))
            C1 = 6.28125
            C2 = float(2.0 * np.pi - 6.28125)
            TWO_PI = float(2.0 * np.pi)

            def reduce_to_pi(x):
                # x >= 0 (SBUF or PSUM AP) -> SBUF f32 in [-pi, pi]
                t = setup_p.tile([32, SB], F32, tag="rr_t")
                nc.vector.tensor_scalar_mul(t, x, INV2PI)
                ri = setup_p.tile([32, SB], dt.int32, tag="rr_i")
                nc.vector.tensor_copy(ri, t)
                rf = setup_p.tile([32, SB], F32, tag="rr_f")
                nc.vector.tensor_copy(rf, ri)
                a1 = setup_p.tile([32, SB], F32, tag="rr_a1")
                nc.vector.scalar_tensor_tensor(
                    a1, rf, -C1, x,
                    op0=mybir.AluOpType.mult, op1=mybir.AluOpType.add,
                )
                a2 = setup_p.tile([32, SB], F32, tag="rr_a2")
                nc.vector.scalar_tensor_tensor(
                    a2, rf, -C2, a1,
                    op0=mybir.AluOpType.mult, op1=mybir.AluOpType.add,
                )
                over = setup_p.tile([32, SB], F32, tag="rr_ov")
                nc.vector.tensor_scalar(
                    over, a2, float(np.pi), None, op0=mybir.AluOpType.is_gt
                )
                a3 = setup_p.tile([32, SB], F32, tag="rr_a1")
                nc.vector.scalar_tensor_tensor(
                    a3, over, -TWO_PI, a2,
                    op0=mybir.AluOpType.mult, op1=mybir.AluOpType.add,
                )
                return a3

            for cchunk in range(NSB):
                csl = slice(cchunk * SB, (cchunk + 1) * SB)
                ang = ps_p.tile([128, SB], F32, tag="ps")
                # angles = outer(invf, pos) via K=1 fp32 matmul
                nc.tensor.matmul(
                    ang[0:32, :], lhsT=invf, rhs=pos_f[:, csl], start=True, stop=True
                )
                angv = ang[0:32, :]
                a_s = reduce_to_pi(angv)
                nc.scalar.activation(sin32[:, csl], a_s, AF.Sin)
                nc.scalar.activation(nsin32[:, csl], a_s, AF.Sin, scale=-1.0)
                shifted = setup_p.tile([32, SB], F32, tag="rr_sh")
                nc.vector.tensor_scalar_add(shifted, angv, float(np.pi / 2))
                a_c = reduce_to_pi(shifted)
                nc.scalar.activation(cos32[:, csl], a_c, AF.Sin)
            # head-major pair layout: rows [a_h0, b_h0, a_h1, b_h1]
            # cosD [128, S] = cos x4 ; sinPM [128, S] = [-sin; +sin; -sin; +sin]
            cosD = consts.tile([128, S], BF, tag="cosD")
            sinPM = consts.tile([128, S], BF, tag="sinPM")
            for r in range(4):
                nc.sync.dma_start(out=cosD[32 * r : 32 * (r + 1), :], in_=cos32)
            nc.sync.dma_start(out=sinPM[0:32, :], in_=nsin32)
            nc.sync.dma_start(out=sinPM[32:64, :], in_=sin32)
            nc.sync.dma_start(out=sinPM[64:96, :], in_=nsin32)
            nc.sync.dma_start(out=sinPM[96:128, :], in_=sin32)

            # ---------------- causal triangle mask [128, 2, 128] ----------------
            # applied to the first 128-col strip of the exp'd region of
            # diagonal tiles: keep iff f' - p >= 0
            tri = consts.tile([128, 2, 128], BF, tag="tri")
            nc.gpsimd.memset(tri, 1.0)
            nc.gpsimd.affine_select(
                out=tri,
                in_=tri,
                compare_op=mybir.AluOpType.is_ge,
                fill=0.0,
                base=0,
                pattern=[[0, 2], [1, 128]],
                channel_multiplier=-1,
            )

            # ---------------- main body (x reps for timing) ----------------
            for rep in range(reps):
                # x^T: [128, 8, S] bf16, 4 chunk DMAs
                xt_t = xt_p.tile([128, 8, S], BF, tag="xt")
                for xc in range(4):
                    xsl = slice(xc * SB, (xc + 1) * SB)
                    nc.sync.dma_start(
                        out=xt_t[:, :, xsl],
                        in_=xt_d.rearrange("t p s -> p t s")[:, :, xsl],
                    )

                # ---- V projection (natural [s, d] per pair + ones cols) ----
                # v_sb [128 s, kt, pair, 130]; cols 64/129 of each 130-block = 1
                v_sb = v_p.tile([128, NKT, NPAIR, 130], BF, tag="v")
                nc.vector.memset(v_sb[:, :, :, 64:65], 1.0)
                nc.vector.memset(v_sb[:, :, :, 129:130], 1.0)
                for kt in range(NKT):
                    vps = ps_p.tile([128, SB], F32, tag="ps")
                    for mt in range(8):
                        nc.tensor.matmul(
                            vps,
                            lhsT=xt_t[:, mt, 128 * kt : 128 * (kt + 1)],
                            rhs=wv_sb[:, mt, :],
                            start=(mt == 0),
                            stop=(mt == 7),
                        )
                    # one strided drain: [p, pair, 2, 64] <- [p, (pair 2 64)]
                    nc.vector.tensor_copy(
                        v_sb[:, kt, :, :].rearrange("p a (b c) -> p a b c", b=2)[
                            :, :, :, 0:64
                        ],
                        vps.rearrange("p (a b c) -> p a b c", a=NPAIR, b=2),
                    )

                # ---- Q^T / K^T projections + RoPE (per pair) ----
                qr = qk_p.tile([128, NPAIR, S], BF, tag="qr")
                kr = qk_p.tile([128, NPAIR, S], BF, tag="kr")
                for (w_sb, dst) in ((wq_sb, qr), (wk_sb, kr)):
                    for pp in range(NPAIR):
                        dsl = slice(128 * pp, 128 * (pp + 1))
                        for sb_i in range(NSB):
                            ssl = slice(sb_i * SB, (sb_i + 1) * SB)
                            tps = ps_p.tile([128, SB], F32, tag="ps")
                            for mt in range(8):
                                nc.tensor.matmul(
                                    tps,
                                    lhsT=w_sb[:, mt, dsl],
                                    rhs=xt_t[:, mt, ssl],
                                    start=(mt == 0),
                                    stop=(mt == 7),
                                )
                            tsb = rope_p.tile([128, SB], BF, tag="tsb")
                            nc.scalar.activation(tsb, tps, AF.Copy)
                            # partner swap within each head: a<->b 32-blocks
                            tswap = rope_p.tile([128, SB], BF, tag="tswap")
                            for h0 in (0, 64):
                                nc.sync.dma_start(
                                    out=tswap[h0 : h0 + 32, :],
                                    in_=tsb[h0 + 32 : h0 + 64, :],
                                )
                                nc.sync.dma_start(
                                    out=tswap[h0 + 32 : h0 + 64, :],
                                    in_=tsb[h0 : h0 + 32, :],
                                )
                            tcos = rope_p.tile([128, SB], BF, tag="tcos")
                            nc.vector.tensor_mul(tcos, tsb, cosD[:, ssl])
                            tsin = rope_p.tile([128, SB], BF, tag="tsin")
                            nc.gpsimd.tensor_mul(tsin, tswap, sinPM[:, ssl])
                            nc.vector.tensor_add(dst[:, pp, ssl], tcos, tsin)

                # ---- attention + deferred output projection ----
                def outproj(qb, ysb):
                    for jj in range(4):
                        qsl2 = slice(128 * jj, 128 * (jj + 1))
                        for mc in range(2):
                            msl = slice(512 * mc, 512 * (mc + 1))
                            ops = ps_p.tile([128, SB], F32, tag="ps")
                            for pp in range(NPAIR):
                                nc.tensor.matmul(
                                    ops,
                                    lhsT=ysb[:, pp, qsl2],
                                    rhs=wo_sb[:, pp, msl],
                                    start=(pp == 0),
                                    stop=(pp == NPAIR - 1),
                                )
                            osb = outs_p.tile([128, SB], BF, tag="osb", bufs=3)
                            nc.vector.tensor_copy(osb, ops)
                            nc.sync.dma_start(
                                out=out_d[
                                    qb * SB + 128 * jj : qb * SB + 128 * (jj + 1),
                                    msl,
                                ],
                                in_=osb,
                            )

                def norm_pair(pp, qb, y0, y1, ysb):
                    # l rows -> partition-0 tiles (gpsimd ucode ignores AP
                    # partition bases), broadcast on gpsimd, then approx-
                    # reciprocal across 64 lanes
                    l0t = lin_p.tile([1, SB], F32, tag="l0t")
                    l1t = lin_p.tile([1, SB], F32, tag="l1t")
                    nc.vector.tensor_copy(l0t, y0[64:65, :])
                    nc.vector.tensor_copy(l1t, y1[64:65, :])
                    lb0r = lin_p.tile([64, SB], F32, tag="lb0r")
                    lb1r = lin_p.tile([64, SB], F32, tag="lb1r")
                    nc.gpsimd.partition_broadcast(lb0r, l0t)
                    nc.gpsimd.partition_broadcast(lb1r, l1t)
                    with nc.allow_low_precision("softmax 1/l"):
                        nc.vector.reciprocal_approx_fast(lb0r, lb0r)
                        nc.vector.reciprocal_approx_fast(lb1r, lb1r)
                    nc.vector.tensor_mul(ysb[0:64, pp, :], y0[0:64, :], lb0r)
                    nc.vector.tensor_mul(ysb[64:128, pp, :], y1[0:64, :], lb1r)

                pending_outproj = None
                for qb in range(NSB):
                    qsl = slice(qb * SB, (qb + 1) * SB)
                    nkb = 4 * (qb + 1)
                    ysb = outs_p.tile([128, NPAIR, SB], BF, tag="ysb")
                    for pp in range(NPAIR):
                        y0 = y_p.tile([128, SB], F32, tag="y")
                        y1 = y_p.tile([128, SB], F32, tag="y")
                        for kb in range(nkb):
                            ksl = slice(128 * kb, 128 * (kb + 1))
                            s_t = s_p.tile([128, 2, SB], F32, tag="s")
                            nc.tensor.matmul(
                                s_t[:, 0, :],
                                lhsT=kr[0:64, pp, ksl],
                                rhs=qr[0:64, pp, qsl],
                                start=True,
                                stop=True,
                            )
                            nc.tensor.matmul(
                                s_t[:, 1, :],
                                lhsT=kr[64:128, pp, ksl],
                                rhs=qr[64:128, pp, qsl],
                                start=True,
                                stop=True,
                            )
                            p_t = p_p.tile([128, 2, SB], BF, tag="p")
                            j = kb - 4 * qb
                            if j < 0:
                                # dense tile: exp everything
                                nc.scalar.activation(
                                    p_t, s_t, AF.Exp, scale=EXP_SCALE
                                )
                            else:
                                # diagonal tile: cols < 128j fully masked
                                if j > 0:
                                    nc.gpsimd.memset(p_t[:, :, 0 : 128 * j], 0.0)
                                nc.scalar.activation(
                                    p_t[:, :, 128 * j :],
                                    s_t[:, :, 128 * j :],
                                    AF.Exp,
                                    scale=EXP_SCALE,
                                )
                                # triangular boundary strip
                                nc.vector.tensor_mul(
                                    p_t[:, :, 128 * j : 128 * (j + 1)],
                                    p_t[:, :, 128 * j : 128 * (j + 1)],
                                    tri,
                                )
                            nc.tensor.matmul(
                                y0[0:65, :],
                                lhsT=v_sb[:, kb, pp, 0:65],
                                rhs=p_t[:, 0, :],
                                start=(kb == 0),
                                stop=(kb == nkb - 1),
                            )
                            nc.tensor.matmul(
                                y1[0:65, :],
                                lhsT=v_sb[:, kb, pp, 65:130],
                                rhs=p_t[:, 1, :],
                                start=(kb == 0),
                                stop=(kb == nkb - 1),
                            )
                        norm_pair(pp, qb, y0, y1, ysb)
                        # run the previous q-block's output projection after
                        # this q-block's first pair is queued, so PE never
                        # stalls on the norm chain
                        if pp == 0 and pending_outproj is not None:
                            outproj(*pending_outproj)
                            pending_outproj = None
                    pending_outproj = (qb, ysb)

                if pending_outproj is not None:
                    outproj(*pending_outproj)
                    pending_outproj = None

    nc.compile()
    return nc


_NC_CACHE = {}


def get_nc(reps=1):
    if reps not in _NC_CACHE:
        _NC_CACHE[reps] = _build_nc(reps)
    return _NC_CACHE[reps]


def make_in_maps(x, token_positions, Wq, Wk, Wv, Wo):
    x = np.asarray(x, dtype=np.float32)
    Wq, Wk, Wv, Wo = (np.asarray(w, dtype=np.float32) for w in (Wq, Wk, Wv, Wo))
    pos = np.ascontiguousarray(
        np.asarray(token_positions, dtype=np.int32).reshape(1, S)
    )
    # [B, 8, 128, S] bf16
    xt = np.ascontiguousarray(x.transpose(0, 2, 1)).astype(BF16)
    xt = xt.reshape(B, 8, 128, S)
    in_maps = []
    for c in range(NCORES):
        g, bc = divmod(c, 4)
        # rope row order for the 4 pairs of head group g:
        # per pair 128-block: [a_hA(32), b_hA(32), a_hB(32), b_hB(32)]
        rows = []
        for ppp in range(NPAIR):
            hA = 8 * g + 2 * ppp
            hB = hA + 1
            rows.extend(64 * hA + np.arange(0, 64, 2))
            rows.extend(64 * hA + np.arange(1, 64, 2))
            rows.extend(64 * hB + np.arange(0, 64, 2))
            rows.extend(64 * hB + np.arange(1, 64, 2))
        rows = np.asarray(rows)
        in_maps.append(
            {
                "xt": xt[bc],
                "wqt": np.ascontiguousarray(Wq[rows, :].T).astype(BF16),
                "wkt": np.ascontiguousarray(Wk[rows, :].T).astype(BF16),
                "wvt": np.ascontiguousarray(
                    Wv[512 * g : 512 * (g + 1), :].T
                ).astype(BF16),
                "wot": np.ascontiguousarray(
                    Wo[:, 512 * g : 512 * (g + 1)].T
                ).astype(BF16),
                "pos": pos,
            }
        )
    return in_maps


def kernel(x, token_positions, Wq, Wk, Wv, Wo):
    from concourse.bass_utils import run_bass_kernel_spmd

    nc = get_nc()
    in_maps = make_in_maps(x, token_positions, Wq, Wk, Wv, Wo)
    res = run_bass_kernel_spmd(nc, in_maps, core_ids=list(range(NCORES)))
    out = np.zeros((B, S, DM), np.float32)
    for c in range(NCORES):
        out[c % 4] += res.results[c]["outp"].astype(np.float32)
    return out


# revision 21
# speedup vs baseline: 1.1327x; 1.0187x over previous
"""Multi-head self-attention with RoPE on 8 Trainium2 NeuronCores.

Sharding: 2-way tensor parallel over heads x 4-way data parallel over batch.
Core c handles batch (c % 4) and head group (c // 4) = 8 heads = 4 head-pairs.
Each core computes Q/K/V projections for its 8 heads on its batch, causal
flash-style attention per head-pair (scores computed transposed, row-tiled
across PE quadrants; softmax denominator via a ones-column in V), and a
partial output projection over its 512 rows of Wo. Host sums 2 partials per
batch in f32.

vs the 2-heads x 4-batches sharding this cuts the per-core partial-output
volume (PSUM drain + DMA) by 4x for the same PE work.

Self-contained: hardcodes all shapes from the problem spec.
"""

import numpy as np
import ml_dtypes

BF16 = ml_dtypes.bfloat16

B, S, DM = 4, 2048, 1024
H, DH = 16, 64
NCORES = 8
NPAIR = 4  # head-pairs per core
DL = NPAIR * 2 * DH  # 512 local head dims per core
SB = 512  # q-block width
NSB = S // SB  # 4
NKT = S // 128  # 16 k-tiles
LN_THETA = float(np.log(10000.0))
EXP_SCALE = 0.125

CFG = {
    "S_BUFS": 2,
    "Y_BUFS": 2,
    "PS_BUFS": 2,
    "QK_BUFS": 1,
    "V_BUFS": 1,
    "L_BUFS": 1,
    "P_BUFS": 4,
    "R_BUFS": 3,
    "O_BUFS": 2,
}


def _build_nc(reps=1):
    import concourse.bass as bass
    import concourse.tile as tile
    import concourse.mybir as mybir
    from concourse import bacc

    dt = mybir.dt
    F32 = dt.float32
    BF = dt.bfloat16
    AF = mybir.ActivationFunctionType

    nc = bacc.Bacc("TRN2", target_bir_lowering=False, debug=False)

    xt_d = nc.dram_tensor("xt", [DM // 128, 128, S], BF, kind="ExternalInput").ap()
    wq_d = nc.dram_tensor("wqt", [DM, DL], BF, kind="ExternalInput").ap()
    wk_d = nc.dram_tensor("wkt", [DM, DL], BF, kind="ExternalInput").ap()
    wv_d = nc.dram_tensor("wvt", [DM, DL], BF, kind="ExternalInput").ap()
    wo_d = nc.dram_tensor("wot", [DL, DM], BF, kind="ExternalInput").ap()
    pos_d = nc.dram_tensor("pos", [1, S], dt.int32, kind="ExternalInput").ap()
    out_d = nc.dram_tensor("outp", [S, DM], BF, kind="ExternalOutput").ap()

    with tile.TileContext(nc) as tc:
        import contextlib

        ctx = contextlib.ExitStack()
        with ctx:
            # ---------------- pools ----------------
            consts = ctx.enter_context(tc.tile_pool(name="consts", bufs=1))
            xt_p = ctx.enter_context(tc.tile_pool(name="xt", bufs=1))
            qk_p = ctx.enter_context(tc.tile_pool(name="qk", bufs=CFG["QK_BUFS"]))
            rope_p = ctx.enter_context(tc.tile_pool(name="rope", bufs=CFG["R_BUFS"]))
            setup_p = ctx.enter_context(tc.tile_pool(name="setup", bufs=1))
            v_p = ctx.enter_context(tc.tile_pool(name="v", bufs=CFG["V_BUFS"]))
            p_p = ctx.enter_context(tc.tile_pool(name="p", bufs=CFG["P_BUFS"]))
            lin_p = ctx.enter_context(tc.tile_pool(name="lin", bufs=CFG["L_BUFS"]))
            outs_p = ctx.enter_context(tc.tile_pool(name="outs", bufs=CFG["O_BUFS"]))
            # shared [128, 512] f32 PSUM pool: projections, angle outer-
            # products, and output projection (phases don't overlap much)
            ps_p = ctx.enter_context(
                tc.tile_pool(name="ps", bufs=CFG["PS_BUFS"], space="PSUM")
            )
            s_p = ctx.enter_context(
                tc.tile_pool(name="s", bufs=CFG["S_BUFS"], space="PSUM")
            )
            y_p = ctx.enter_context(
                tc.tile_pool(name="y", bufs=CFG["Y_BUFS"], space="PSUM")
            )

            # ---------------- weights -> SBUF ----------------
            # pos first: the table build (PE's first work) needs only pos
            pos_i = consts.tile([1, S], dt.int32, tag="pos_i")
            nc.sync.dma_start(out=pos_i, in_=pos_d)
            wq_sb = consts.tile([128, 8, DL], BF, tag="wq")
            wk_sb = consts.tile([128, 8, DL], BF, tag="wk")
            wv_sb = consts.tile([128, 8, DL], BF, tag="wv")
            wo_sb = consts.tile([128, NPAIR, DM], BF, tag="wo")
            # wk/wq now; wv/wo after the first xt chunk (emitted in the rep
            # loop) so the in-order SP queue feeds qk_proj(0,0) earliest
            nc.sync.dma_start(out=wk_sb, in_=wk_d.rearrange("(t p) d -> p t d", p=128))
            nc.sync.dma_start(out=wq_sb, in_=wq_d.rearrange("(t p) d -> p t d", p=128))

            # ---------------- cos/sin tables ----------------
            # invf row [1, 32]: exp(-j * 2*ln(theta)/64)
            invf_i = consts.tile([1, 32], dt.int32, tag="invf_i")
            nc.gpsimd.iota(invf_i, pattern=[[1, 32]], base=0, channel_multiplier=0)
            invf_f = consts.tile([1, 32], F32, tag="invf_f")
            nc.vector.tensor_copy(invf_f, invf_i)
            invf = consts.tile([1, 32], F32, tag="invf")
            nc.scalar.activation(invf, invf_f, AF.Exp, scale=-(2.0 * LN_THETA / 64.0))
            pos_f = consts.tile([1, S], F32, tag="pos_f")
            nc.vector.tensor_copy(pos_f, pos_i)

            sin32 = consts.tile([32, S], BF, tag="sin32")
            nsin32 = consts.tile([32, S], BF, tag="nsin32")
            cos32 = consts.tile([32, S], BF, tag="cos32")
            # Sin LUT needs args in [-pi, pi]: Cody-Waite range reduction.
            # HW f32->i32 conversion rounds to nearest; CoreSim truncates.
            # The is_gt fix-up makes the result exact under both (args >= 0).
            INV2PI = float(1.0 / (2.0 * np.pi))
            C1 = 6.28125
            C2 = float(2.0 * np.pi - 6.28125)
            TWO_PI = float(2.0 * np.pi)

            def reduce_to_pi(x):
                # x >= 0 (SBUF or PSUM AP) -> SBUF f32 in [-pi, pi]
                t = setup_p.tile([32, SB], F32, tag="rr_t")
                nc.vector.tensor_scalar_mul(t, x, INV2PI)
                ri = setup_p.tile([32, SB], dt.int32, tag="rr_i")
                nc.vector.tensor_copy(ri, t)
                rf = setup_p.tile([32, SB], F32, tag="rr_f")
                nc.vector.tensor_copy(rf, ri)
                a1 = setup_p.tile([32, SB], F32, tag="rr_a1")
                nc.vector.scalar_tensor_tensor(
                    a1, rf, -C1, x,
                    op0=mybir.AluOpType.mult, op1=mybir.AluOpType.add,
                )
                a2 = setup_p.tile([32, SB], F32, tag="rr_a2")
                nc.vector.scalar_tensor_tensor(
                    a2, rf, -C2, a1,
                    op0=mybir.AluOpType.mult, op1=mybir.AluOpType.add,
                )
                over = setup_p.tile([32, SB], F32, tag="rr_ov")
                nc.vector.tensor_scalar(
                    over, a2, float(np.pi), None, op0=mybir.AluOpType.is_gt
                )
                a3 = setup_p.tile([32, SB], F32, tag="rr_a1")
                nc.vector.scalar_tensor_tensor(
                    a3, over, -TWO_PI, a2,
                    op0=mybir.AluOpType.mult, op1=mybir.AluOpType.add,
                )
                return a3

            for cchunk in range(NSB):
                csl = slice(cchunk * SB, (cchunk + 1) * SB)
                # borrow the y PSUM pool (idle during setup) so the table
                # build never blocks the projection pipeline's ps bufs
                ang = y_p.tile([128, SB], F32, tag="y")
                # angles = outer(invf, pos) via K=1 fp32 matmul
                nc.tensor.matmul(
                    ang[0:32, :], lhsT=invf, rhs=pos_f[:, csl], start=True, stop=True
                )
                angv = ang[0:32, :]
                a_s = reduce_to_pi(angv)
                nc.scalar.activation(sin32[:, csl], a_s, AF.Sin)
                nc.scalar.activation(nsin32[:, csl], a_s, AF.Sin, scale=-1.0)
                shifted = setup_p.tile([32, SB], F32, tag="rr_sh")
                nc.vector.tensor_scalar_add(shifted, angv, float(np.pi / 2))
                a_c = reduce_to_pi(shifted)
                nc.scalar.activation(cos32[:, csl], a_c, AF.Sin)
            # head-major pair layout: rows [a_h0, b_h0, a_h1, b_h1]
            # cosD [128, S] = cos x4 ; sinPM [128, S] = [-sin; +sin; -sin; +sin]
            # assembly copies go on the Activation HWDGE queue: they wait on
            # computed sin/cos, and on the in-order SP queue they would
            # head-of-line block the xt loads behind them
            cosD = consts.tile([128, S], BF, tag="cosD")
            sinPM = consts.tile([128, S], BF, tag="sinPM")
            for r in range(4):
                nc.scalar.dma_start(out=cosD[32 * r : 32 * (r + 1), :], in_=cos32)
            nc.scalar.dma_start(out=sinPM[0:32, :], in_=nsin32)
            nc.scalar.dma_start(out=sinPM[32:64, :], in_=sin32)
            nc.scalar.dma_start(out=sinPM[64:96, :], in_=nsin32)
            nc.scalar.dma_start(out=sinPM[96:128, :], in_=sin32)

            # ---------------- causal triangle mask [128, 2, 128] ----------------
            # applied to the first 128-col strip of the exp'd region of
            # diagonal tiles: keep iff f' - p >= 0
            tri = consts.tile([128, 2, 128], BF, tag="tri")
            nc.gpsimd.memset(tri, 1.0)
            nc.gpsimd.affine_select(
                out=tri,
                in_=tri,
                compare_op=mybir.AluOpType.is_ge,
                fill=0.0,
                base=0,
                pattern=[[0, 2], [1, 128]],
                channel_multiplier=-1,
            )

            # ---------------- main body (x reps for timing) ----------------
            for rep in range(reps):
                # x^T: [128, 8, S] bf16, 4 chunk DMAs
                xt_t = xt_p.tile([128, 8, S], BF, tag="xt")
                for xc in range(4):
                    xsl = slice(xc * SB, (xc + 1) * SB)
                    nc.sync.dma_start(
                        out=xt_t[:, :, xsl],
                        in_=xt_d.rearrange("t p s -> p t s")[:, :, xsl],
                    )
                    if rep == 0 and xc == 0:
                        nc.sync.dma_start(
                            out=wv_sb, in_=wv_d.rearrange("(t p) d -> p t d", p=128)
                        )
                        nc.sync.dma_start(
                            out=wo_sb, in_=wo_d.rearrange("(t p) m -> p t m", p=128)
                        )

                # ---- V projection (natural [s, d] per pair + ones cols) ----
                # v_sb [128 s, kt, pair, 130]; cols 64/129 of each 130-block = 1
                v_sb = v_p.tile([128, NKT, NPAIR, 130], BF, tag="v")
                nc.vector.memset(v_sb[:, :, :, 64:65], 1.0)
                nc.vector.memset(v_sb[:, :, :, 129:130], 1.0)
                qr = qk_p.tile([128, NPAIR, S], BF, tag="qr")
                kr = qk_p.tile([128, NPAIR, S], BF, tag="kr")

                def v_group(kg):
                    # V projection for k-tiles 4*kg .. 4*kg+3
                    for kt in range(4 * kg, 4 * kg + 4):
                        vps = ps_p.tile([128, SB], F32, tag="ps")
                        for mt in range(8):
                            nc.tensor.matmul(
                                vps,
                                lhsT=xt_t[:, mt, 128 * kt : 128 * (kt + 1)],
                                rhs=wv_sb[:, mt, :],
                                start=(mt == 0),
                                stop=(mt == 7),
                            )
                        # one strided drain: [p, pair, 2, 64] <- [p, (pair 2 64)]
                        nc.vector.tensor_copy(
                            v_sb[:, kt, :, :].rearrange(
                                "p a (b c) -> p a b c", b=2
                            )[:, :, :, 0:64],
                            vps.rearrange("p (a b c) -> p a b c", a=NPAIR, b=2),
                        )

                def qk_proj(sb_i, pp):
                    # K then Q projection + RoPE for seq chunk sb_i, pair pp
                    ssl = slice(sb_i * SB, (sb_i + 1) * SB)
                    dsl = slice(128 * pp, 128 * (pp + 1))
                    for (w_sb, dst) in ((wk_sb, kr), (wq_sb, qr)):
                        tps = ps_p.tile([128, SB], F32, tag="ps")
                        for mt in range(8):
                            nc.tensor.matmul(
                                tps,
                                lhsT=w_sb[:, mt, dsl],
                                rhs=xt_t[:, mt, ssl],
                                start=(mt == 0),
                                stop=(mt == 7),
                            )
                        tsb = rope_p.tile([128, SB], BF, tag="tsb")
                        nc.scalar.activation(tsb, tps, AF.Copy)
                        # partner swap within each head: a<->b 32-blocks
                        tswap = rope_p.tile([128, SB], BF, tag="tswap")
                        for h0 in (0, 64):
                            nc.sync.dma_start(
                                out=tswap[h0 : h0 + 32, :],
                                in_=tsb[h0 + 32 : h0 + 64, :],
                            )
                            nc.sync.dma_start(
                                out=tswap[h0 + 32 : h0 + 64, :],
                                in_=tsb[h0 : h0 + 32, :],
                            )
                        tcos = rope_p.tile([128, SB], BF, tag="tcos")
                        nc.vector.tensor_mul(tcos, tsb, cosD[:, ssl])
                        tsin = rope_p.tile([128, SB], BF, tag="tsin")
                        nc.gpsimd.tensor_mul(tsin, tswap, sinPM[:, ssl])
                        nc.vector.tensor_add(dst[:, pp, ssl], tcos, tsin)

                # ---- attention + deferred output projection ----
                def outproj(qb, ysb):
                    for jj in range(4):
                        qsl2 = slice(128 * jj, 128 * (jj + 1))
                        for mc in range(2):
                            msl = slice(512 * mc, 512 * (mc + 1))
                            ops = ps_p.tile([128, SB], F32, tag="ps")
                            for pp in range(NPAIR):
                                nc.tensor.matmul(
                                    ops,
                                    lhsT=ysb[:, pp, qsl2],
                                    rhs=wo_sb[:, pp, msl],
                                    start=(pp == 0),
                                    stop=(pp == NPAIR - 1),
                                )
                            osb = outs_p.tile([128, SB], BF, tag="osb", bufs=3)
                            nc.vector.tensor_copy(osb, ops)
                            # Act HWDGE queue: waits on the drain, would HOL-
                            # block rope swaps / xt loads on the SP queue
                            nc.scalar.dma_start(
                                out=out_d[
                                    qb * SB + 128 * jj : qb * SB + 128 * (jj + 1),
                                    msl,
                                ],
                                in_=osb,
                            )

                def norm_pair(pp, qb, y0, y1, ysb):
                    # Copy l rows + y dims to SBUF first so the y PSUM banks
                    # free after ~1.2us instead of after the whole chain
                    # (next pair's AV start=True waits on these buffers).
                    # l rows go to partition-0 tiles (gpsimd ucode ignores AP
                    # partition bases), broadcast on gpsimd, then approx-
                    # reciprocal across 64 lanes.
                    l0t = lin_p.tile([1, SB], F32, tag="l0t")
                    l1t = lin_p.tile([1, SB], F32, tag="l1t")
                    nc.vector.tensor_copy(l0t, y0[64:65, :])
                    nc.vector.tensor_copy(l1t, y1[64:65, :])
                    # two base-0 tiles: SB+SB tensor ops need equal input bases
                    yc0 = lin_p.tile([64, SB], F32, tag="yc0")
                    yc1 = lin_p.tile([64, SB], F32, tag="yc1")
                    nc.vector.tensor_copy(yc0, y0[0:64, :])
                    nc.vector.tensor_copy(yc1, y1[0:64, :])
                    lb0r = lin_p.tile([64, SB], F32, tag="lb0r")
                    lb1r = lin_p.tile([64, SB], F32, tag="lb1r")
                    nc.gpsimd.partition_broadcast(lb0r, l0t)
                    nc.gpsimd.partition_broadcast(lb1r, l1t)
                    with nc.allow_low_precision("softmax 1/l"):
                        nc.vector.reciprocal_approx_fast(lb0r, lb0r)
                        nc.vector.reciprocal_approx_fast(lb1r, lb1r)
                    nc.vector.tensor_mul(ysb[0:64, pp, :], yc0, lb0r)
                    nc.vector.tensor_mul(ysb[64:128, pp, :], yc1, lb1r)

                pending_outproj = None
                for qb in range(NSB):
                    # pipeline: projections for chunk qb feed attention qb
                    # (qb attends keys < 512*(qb+1) and queries in chunk qb),
                    # interleaved per pair so attention(qb, pp) starts right
                    # after pair pp's rope while other pairs still project;
                    # PE's proj surplus covers the Act-bound attention phase
                    qsl = slice(qb * SB, (qb + 1) * SB)
                    nkb = 4 * (qb + 1)
                    ysb = outs_p.tile([128, NPAIR, SB], BF, tag="ysb")
                    for pp in range(NPAIR):
                        qk_proj(qb, pp)
                        if pp == 0:
                            v_group(qb)
                        y0 = y_p.tile([128, SB], F32, tag="y")
                        y1 = y_p.tile([128, SB], F32, tag="y")
                        for kb in range(nkb):
                            ksl = slice(128 * kb, 128 * (kb + 1))
                            s_t = s_p.tile([128, 2, SB], F32, tag="s")
                            nc.tensor.matmul(
                                s_t[:, 0, :],
                                lhsT=kr[0:64, pp, ksl],
                                rhs=qr[0:64, pp, qsl],
                                start=True,
                                stop=True,
                            )
                            nc.tensor.matmul(
                                s_t[:, 1, :],
                                lhsT=kr[64:128, pp, ksl],
                                rhs=qr[64:128, pp, qsl],
                                start=True,
                                stop=True,
                            )
                            p_t = p_p.tile([128, 2, SB], BF, tag="p")
                            j = kb - 4 * qb
                            if j < 0:
                                # dense tile: exp everything
                                nc.scalar.activation(
                                    p_t, s_t, AF.Exp, scale=EXP_SCALE
                                )
                            else:
                                # diagonal tile: cols < 128j fully masked
                                if j > 0:
                                    nc.gpsimd.memset(p_t[:, :, 0 : 128 * j], 0.0)
                                nc.scalar.activation(
                                    p_t[:, :, 128 * j :],
                                    s_t[:, :, 128 * j :],
                                    AF.Exp,
                                    scale=EXP_SCALE,
                                )
                                # triangular boundary strip
                                nc.vector.tensor_mul(
                                    p_t[:, :, 128 * j : 128 * (j + 1)],
                                    p_t[:, :, 128 * j : 128 * (j + 1)],
                                    tri,
                                )
                            nc.tensor.matmul(
                                y0[0:65, :],
                                lhsT=v_sb[:, kb, pp, 0:65],
                                rhs=p_t[:, 0, :],
                                start=(kb == 0),
                                stop=(kb == nkb - 1),
                            )
                            nc.tensor.matmul(
                                y1[0:65, :],
                                lhsT=v_sb[:, kb, pp, 65:130],
                                rhs=p_t[:, 1, :],
                                start=(kb == 0),
                                stop=(kb == nkb - 1),
                            )
                        norm_pair(pp, qb, y0, y1, ysb)
                        # run the previous q-block's output projection after
                        # this q-block's first pair is queued, so PE never
                        # stalls on the norm chain
                        if pp == 0 and pending_outproj is not None:
                            outproj(*pending_outproj)
                            pending_outproj = None
                    pending_outproj = (qb, ysb)

                if pending_outproj is not None:
                    outproj(*pending_outproj)
                    pending_outproj = None

    nc.compile()
    return nc


_NC_CACHE = {}


def get_nc(reps=1):
    if reps not in _NC_CACHE:
        _NC_CACHE[reps] = _build_nc(reps)
    return _NC_CACHE[reps]


def make_in_maps(x, token_positions, Wq, Wk, Wv, Wo):
    x = np.asarray(x, dtype=np.float32)
    Wq, Wk, Wv, Wo = (np.asarray(w, dtype=np.float32) for w in (Wq, Wk, Wv, Wo))
    pos = np.ascontiguousarray(
        np.asarray(token_positions, dtype=np.int32).reshape(1, S)
    )
    # [B, 8, 128, S] bf16
    xt = np.ascontiguousarray(x.transpose(0, 2, 1)).astype(BF16)
    xt = xt.reshape(B, 8, 128, S)
    in_maps = []
    for c in range(NCORES):
        g, bc = divmod(c, 4)
        # rope row order for the 4 pairs of head group g:
        # per pair 128-block: [a_hA(32), b_hA(32), a_hB(32), b_hB(32)]
        rows = []
        for ppp in range(NPAIR):
            hA = 8 * g + 2 * ppp
            hB = hA + 1
            rows.extend(64 * hA + np.arange(0, 64, 2))
            rows.extend(64 * hA + np.arange(1, 64, 2))
            rows.extend(64 * hB + np.arange(0, 64, 2))
            rows.extend(64 * hB + np.arange(1, 64, 2))
        rows = np.asarray(rows)
        in_maps.append(
            {
                "xt": xt[bc],
                "wqt": np.ascontiguousarray(Wq[rows, :].T).astype(BF16),
                "wkt": np.ascontiguousarray(Wk[rows, :].T).astype(BF16),
                "wvt": np.ascontiguousarray(
                    Wv[512 * g : 512 * (g + 1), :].T
                ).astype(BF16),
                "wot": np.ascontiguousarray(
                    Wo[:, 512 * g : 512 * (g + 1)].T
                ).astype(BF16),
                "pos": pos,
            }
        )
    return in_maps


def kernel(x, token_positions, Wq, Wk, Wv, Wo):
    from concourse.bass_utils import run_bass_kernel_spmd

    nc = get_nc()
    in_maps = make_in_maps(x, token_positions, Wq, Wk, Wv, Wo)
    res = run_bass_kernel_spmd(nc, in_maps, core_ids=list(range(NCORES)))
    out = np.zeros((B, S, DM), np.float32)
    for c in range(NCORES):
        out[c % 4] += res.results[c]["outp"].astype(np.float32)
    return out
